# revision 10
# baseline (speedup 1.0000x reference)
"""CosFormer attention (causal, relu, out_proj) — Trainium2 Bass kernel, 8-core SPMD.

Sharding: the 16 (batch*head) slices are split 2-per-core across 8 cores
(cores 0-3 -> batch 0 heads 0-7, cores 4-7 -> batch 1). Each core runs a
chunked linear-attention scan (chunk=256, sub-chunk=128) for its 2 heads plus
a partial out-projection over its 128 local features; the host sums the 4
partials per batch and adds the bias (pure unshard work).

Device math per head (feature-major layouts, l/j = time, d = 2*HID feature):
  Qf[d,l] = qT*sin / qT*cos (host-scaled const diag), relu on device -> Q
  A^T[j,l] = K^T Q  (per 128-row strip), causal-masked on DVE
  qkv^T(+denom row) = va^T @ A^T_masked + S^T_prev-ish via S state matmuls
  attn^T = qkv^T * bcast(1/denom);  out^T += wloc^T-style matmul per e-chunk
"""

import os
from contextlib import ExitStack

import numpy as np

import concourse.bass as bass
import concourse.tile as tile
from concourse import bacc
from concourse import mybir
from concourse.bass_utils import run_bass_kernel_spmd
from concourse.masks import make_upper_triangular

N, H, T, HID = 2, 8, 1024, 64
NH = N * H
D2 = 2 * HID          # 128 feature dim after sin/cos concat
C = 256               # chunk length
SC = 128              # sub-chunk (matmul strip) length
NCH = T // C          # 4 chunks per head
NSC = T // SC         # 8 sub-chunks per head
E = 512               # embed dim
EPS = 1e-6
CORES = 8

F32 = mybir.dt.float32
F32R = mybir.dt.float32r

# fp32r (1 cyc/row at N>=256) vs full fp32 (4 cyc/row) for matmul inputs.
# The BIR verifier requires fp32r matmul operands to be *produced* as fp32r,
# so matmul-feeding tensors are declared float32r end to end.
MM_DT = os.environ.get("COS_MM_DT", "f32r")
DT_IN = F32R if MM_DT == "f32r" else F32


def _mm(ap):
    return ap


def _build_bass():
    nc = bacc.Bacc(None, target_bir_lowering=False)

    qf = nc.dram_tensor("qf", (2, D2, T), F32, kind="ExternalInput")
    kf = nc.dram_tensor("kf", (2, D2, T), F32, kind="ExternalInput")
    kt = nc.dram_tensor("kt", (2, NSC, SC, D2), F32, kind="ExternalInput")
    va = nc.dram_tensor("va", (2, NSC, SC, 256), DT_IN, kind="ExternalInput")
    wloc = nc.dram_tensor("wloc", (D2, E), DT_IN, kind="ExternalInput")
    osel = nc.dram_tensor("osel", (1, 2, D2), DT_IN, kind="ExternalInput")
    outp = nc.dram_tensor("outp", (E, T), F32, kind="ExternalOutput")

    with ExitStack() as ctx:
        ctx.enter_context(nc.allow_low_precision(reason="fp32r matmul inputs"))
        tc = ctx.enter_context(tile.TileContext(nc))
        consts = ctx.enter_context(tc.tile_pool(name="consts", bufs=1))
        io = ctx.enter_context(tc.tile_pool(name="io", bufs=2))
        work = ctx.enter_context(tc.tile_pool(name="work", bufs=2))
        psum = ctx.enter_context(tc.tile_pool(name="psum", bufs=2, space="PSUM"))

        # --- constants built on device ---
        # combined causal mask for one chunk's two scorestrips:
        # cols 0-255: strip sc0 = [UT | ones];  cols 256-511: strip sc1 = [zeros | UT]
        maskc = consts.tile([SC, 512], F32)
        nc.gpsimd.memset(maskc[:, 128:256], 1.0)
        nc.gpsimd.memset(maskc[:, 256:384], 0.0)
        make_upper_triangular(nc, maskc[:, 0:128], val=1.0, diag=True)
        make_upper_triangular(nc, maskc[:, 384:512], val=1.0, diag=True)

        # broadcast selectors (1 partition x 128): head slot 0 -> rows 0-63,
        # slot 1 -> rows 64-127 of the broadcast matmul output
        ones_sel = consts.tile([1, 2, D2], DT_IN)
        nc.sync.dma_start(out=ones_sel, in_=osel[:, :, :])

        wloc_sb = consts.tile([D2, E], DT_IN)
        nc.sync.dma_start(out=wloc_sb, in_=wloc[:, :])

        attnT = consts.tile([D2, T], DT_IN)  # both heads' attn^T (f-major)

        for h in range(2):
            qrow = slice(0, 64) if h == 0 else slice(64, 128)
            drow = 64 if h == 0 else 32

            # --- loads ---
            qf_sb = io.tile([D2, T], F32, tag="qf")
            nc.sync.dma_start(out=qf_sb, in_=qf[h])
            kf_sb = io.tile([D2, T], F32, tag="kf")
            nc.sync.dma_start(out=kf_sb, in_=kf[h])
            kt_sb = io.tile([SC, NSC, D2], F32, tag="kt")
            nc.sync.dma_start(out=kt_sb, in_=kt[h].rearrange("s j d -> j s d"))
            # va host-padded to 256 cols per sub-chunk so the state matmul rhs
            # is N=256 (fp32r full rate); cols 128-255 are zero
            va_sb = io.tile([SC, NSC, 256], DT_IN, tag="va")
            nc.sync.dma_start(out=va_sb, in_=va[h].rearrange("s j d -> j s d"))

            # --- relu (q/k only; v passes through raw) ---
            Qr = work.tile([D2, T], DT_IN, tag="Qr")
            nc.scalar.activation(Qr, qf_sb, mybir.ActivationFunctionType.Relu)
            Kr = work.tile([D2, T], DT_IN, tag="Kr")
            nc.vector.tensor_scalar_max(Kr, kf_sb, 0.0)
            Ktr = work.tile([SC, NSC, D2], DT_IN, tag="Ktr")
            nc.gpsimd.tensor_scalar_max(Ktr, kt_sb, 0.0)

            qkvT = work.tile([D2, T], F32, tag="qkvT")
            psS = psum.tile([D2, 256], F32, tag="psS")
            S_sb = None

            for cc in range(NCH):
                lcols = slice(cc * C, (cc + 1) * C)
                # scores: two (128 x 256) strips side by side in one bank
                psA = psum.tile([SC, 512], F32, tag="psA")
                for s2 in range(2):
                    sc = 2 * cc + s2
                    nc.tensor.matmul(
                        psA[:, s2 * 256:(s2 + 1) * 256],
                        _mm(Kr[:, sc * SC:(sc + 1) * SC]),
                        _mm(Qr[:, lcols]),
                        start=True, stop=True,
                    )
                ATm = work.tile([SC, 512], DT_IN, tag="ATm")
                nc.vector.tensor_mul(ATm, psA, maskc)

                # qkv^T (+denom row drow) for this chunk
                psO = psum.tile([D2, C], F32, tag="psO")
                n_mm = 3 if cc > 0 else 2
                for s2 in range(2):
                    sc = 2 * cc + s2
                    nc.tensor.matmul(
                        psO,
                        _mm(va_sb[:, sc, 0:D2]),
                        _mm(ATm[:, s2 * 256:(s2 + 1) * 256]),
                        start=(s2 == 0), stop=(s2 == n_mm - 1),
                    )
                if cc > 0:
                    nc.tensor.matmul(
                        psO, _mm(S_sb), _mm(Qr[:, lcols]), start=False, stop=True,
                    )

                # state update for the *next* chunk
                if cc < NCH - 1:
                    for s2 in range(2):
                        sc = 2 * cc + s2
                        nc.tensor.matmul(
                            psS,
                            _mm(Ktr[:, sc, :]),
                            _mm(va_sb[:, sc, :]),
                            start=(cc == 0 and s2 == 0),
                            stop=(cc == NCH - 2 and s2 == 1),
                            skip_group_check=True,
                        )
                    S_sb = work.tile([D2, D2], DT_IN, tag="S_sb")
                    nc.scalar.copy(S_sb, psS[:, 0:D2])

                nc.scalar.copy(qkvT[:, lcols], psO)

            # --- divide by denominator (row drow of qkvT) ---
            dmax = work.tile([1, T], F32, tag="dmax")
            nc.gpsimd.tensor_scalar_max(dmax, qkvT[drow:drow + 1, :], EPS)
            rr = work.tile([1, T], DT_IN, tag="rr")
            nc.vector.reciprocal(rr, dmax)
            for half in range(2):
                hcols = slice(half * 512, (half + 1) * 512)
                psB = psum.tile([D2, 512], F32, tag="ps512")
                nc.tensor.matmul(
                    psB, _mm(ones_sel[:, h, :]), _mm(rr[:, hcols]),
                    start=True, stop=True,
                )
                nc.vector.tensor_mul(
                    attnT[qrow, hcols], qkvT[qrow, hcols], psB[qrow, :],
                )

        # --- partial out-projection: out^T[e, t] = sum_f wloc[f, e] attnT[f, t] ---
        for ec in range(4):
            ot = work.tile([SC, T], F32, tag="ot")
            for th in range(2):
                psW = psum.tile([SC, 512], F32, tag="ps512")
                nc.tensor.matmul(
                    psW,
                    _mm(wloc_sb[:, ec * SC:(ec + 1) * SC]),
                    _mm(attnT[:, th * 512:(th + 1) * 512]),
                    start=True, stop=True,
                )
                if th == 0:
                    nc.vector.tensor_copy(ot[:, th * 512:(th + 1) * 512], psW)
                else:
                    nc.scalar.copy(ot[:, th * 512:(th + 1) * 512], psW)
            nc.sync.dma_start(out=outp[ec * SC:(ec + 1) * SC, :], in_=ot)

    nc.finalize()
    return nc


_NC_CACHE = {}


def _get_nc():
    key = MM_DT
    if key not in _NC_CACHE:
        _NC_CACHE[key] = _build_bass()
    return _NC_CACHE[key]


def _host_prep(q, k, v):
    """Build per-core input shards (layout/constant-scale prep only)."""
    idx = (np.float32(np.pi / 2) * np.arange(1, T + 1, dtype=np.float32)) / np.float32(T)
    s = np.sin(idx).astype(np.float32)
    c = np.cos(idx).astype(np.float32)

    q3 = q.reshape(NH, T, HID)
    k3 = k.reshape(NH, T, HID)
    vr = v.reshape(T, NH, HID).transpose(1, 0, 2)  # faithful buffer reinterpret

    qT = q3.transpose(0, 2, 1)
    kT = k3.transpose(0, 2, 1)
    qf = np.concatenate([qT * s[None, None, :], qT * c[None, None, :]], axis=1)
    kf = np.concatenate([kT * s[None, None, :], kT * c[None, None, :]], axis=1)
    kt = np.concatenate([k3 * s[:, None], k3 * c[:, None]], axis=-1)
    kt = np.ascontiguousarray(kt.reshape(NH, NSC, SC, D2))

    va = np.zeros((NH, NSC, SC, 256), np.float32)
    vch = vr.reshape(NH, NSC, SC, HID)
    va[0::2, :, :, 0:64] = vch[0::2]
    va[0::2, :, :, 64] = 1.0
    va[1::2, :, :, 32] = 1.0
    va[1::2, :, :, 64:128] = vch[1::2]

    qf = np.ascontiguousarray(qf.reshape(CORES, 2, D2, T))
    kf = np.ascontiguousarray(kf.reshape(CORES, 2, D2, T))
    kt = kt.reshape(CORES, 2, NSC, SC, D2)
    va = va.reshape(CORES, 2, NSC, SC, 256)
    return qf, kf, kt, va


def _assemble(results, W_out, b_out, q_dtype):
    ctx = np.zeros((N, H, T, HID), np.float32)
    for nb in range(N):
        outT = np.zeros((E, T), np.float32)
        for core in range(nb * 4, nb * 4 + 4):
            outT += results[core]["outp"]
        out = outT.T + b_out[None, :]
        ctx[nb] = out.reshape(T, H, HID).transpose(1, 0, 2)
    probs = np.zeros((N, H, T, T), np.float32)
    return ctx.astype(q_dtype, copy=False), probs


def run(q, k, v, W_out, b_out, trace=False, **spmd_kwargs):
    q = np.asarray(q, np.float32)
    k = np.asarray(k, np.float32)
    v = np.asarray(v, np.float32)
    W_out = np.asarray(W_out, np.float32)
    b_out = np.asarray(b_out, np.float32)

    qf, kf, kt, va = _host_prep(q, k, v)
    osel_np = np.zeros((1, 2, D2), np.float32)
    osel_np[0, 0, 0:64] = 1.0
    osel_np[0, 1, 64:128] = 1.0
    in_maps = []
    for core in range(CORES):
        fr = (core % 4) * D2
        in_maps.append({
            "qf": np.ascontiguousarray(qf[core]),
            "kf": np.ascontiguousarray(kf[core]),
            "kt": np.ascontiguousarray(kt[core]),
            "va": np.ascontiguousarray(va[core]),
            "wloc": np.ascontiguousarray(W_out[:, fr:fr + D2].T),
            "osel": osel_np,
        })

    nc = _get_nc()
    res = run_bass_kernel_spmd(nc, in_maps, list(range(CORES)), trace=trace, **spmd_kwargs)
    outputs = _assemble(res.results, W_out, b_out, np.float32)
    return outputs, res


def kernel(q, k, v, W_out, b_out):
    outputs, _ = run(q, k, v, W_out, b_out, trace=False)
    return outputs


# revision 12
# speedup vs baseline: 1.8278x; 1.8278x over previous
"""CosFormer attention (causal, relu, out_proj) — Trainium2 Bass kernel, 8-core SPMD.

Sharding: the 16 (batch*head) slices are split 2-per-core across 8 cores
(cores 0-3 -> batch 0 heads 0-7, cores 4-7 -> batch 1). Each core runs a
chunked linear-attention scan (chunk=256, sub-chunk=128) for its 2 heads plus
a partial out-projection over its 128 local features; the host sums the 4
partials per batch and adds the bias (pure unshard work).

Device math per head (feature-major layouts, l/j = time, d = 2*HID feature):
  Qf[d,l] = qT*sin / qT*cos (host-scaled const diag), relu on device -> Q
  A^T[j,l] = K^T Q  (per 128-row strip), causal-masked on DVE
  qkv^T(+denom row) = va^T @ A^T_masked + S^T_prev-ish via S state matmuls
  attn^T = qkv^T * bcast(1/denom);  out^T += wloc^T-style matmul per e-chunk
"""

import os
from contextlib import ExitStack

import numpy as np

import concourse.bass as bass
import concourse.tile as tile
from concourse import bacc
from concourse import mybir
from concourse.bass_utils import run_bass_kernel_spmd
from concourse.masks import make_upper_triangular

N, H, T, HID = 2, 8, 1024, 64
NH = N * H
D2 = 2 * HID          # 128 feature dim after sin/cos concat
C = 256               # chunk length
SC = 128              # sub-chunk (matmul strip) length
NCH = T // C          # 4 chunks per head
NSC = T // SC         # 8 sub-chunks per head
E = 512               # embed dim
EPS = 1e-6
CORES = 8

F32 = mybir.dt.float32
F32R = mybir.dt.float32r

# fp32r (1 cyc/row at N>=256) vs full fp32 (4 cyc/row) for matmul inputs.
# The BIR verifier requires fp32r matmul operands to be *produced* as fp32r,
# so matmul-feeding tensors are declared float32r end to end.
MM_DT = os.environ.get("COS_MM_DT", "f32r")
DT_IN = F32R if MM_DT == "f32r" else F32


def _mm(ap):
    return ap


def _build_bass():
    nc = bacc.Bacc(None, target_bir_lowering=False)

    qf = nc.dram_tensor("qf", (2, D2, T), F32, kind="ExternalInput")
    kf = nc.dram_tensor("kf", (2, D2, T), F32, kind="ExternalInput")
    kt = nc.dram_tensor("kt", (2, NSC, SC, D2), F32, kind="ExternalInput")
    va = nc.dram_tensor("va", (2, NSC, SC, 256), DT_IN, kind="ExternalInput")
    wloc = nc.dram_tensor("wloc", (D2, E), DT_IN, kind="ExternalInput")
    osel = nc.dram_tensor("osel", (1, 2, D2), F32, kind="ExternalInput")
    outp = nc.dram_tensor("outp", (E, T), F32, kind="ExternalOutput")

    with ExitStack() as ctx:
        ctx.enter_context(nc.allow_low_precision(reason="fp32r matmul inputs"))
        tc = ctx.enter_context(tile.TileContext(nc))
        consts = ctx.enter_context(tc.tile_pool(name="consts", bufs=1))
        io = ctx.enter_context(tc.tile_pool(name="io", bufs=2))
        work = ctx.enter_context(tc.tile_pool(name="work", bufs=2))
        psum = ctx.enter_context(tc.tile_pool(name="psum", bufs=2, space="PSUM"))

        # --- constants built on device ---
        # combined causal mask for one chunk's two scorestrips:
        # cols 0-255: strip sc0 = [UT | ones];  cols 256-511: strip sc1 = [zeros | UT]
        maskc = consts.tile([SC, 512], F32)
        nc.gpsimd.memset(maskc[:, 128:256], 1.0)
        nc.gpsimd.memset(maskc[:, 256:384], 0.0)
        make_upper_triangular(nc, maskc[:, 0:128], val=1.0, diag=True)
        make_upper_triangular(nc, maskc[:, 384:512], val=1.0, diag=True)

        # broadcast selectors (1 partition x 128): head slot 0 -> rows 0-63,
        # slot 1 -> rows 64-127 of the broadcast matmul output
        ones_sel = consts.tile([1, 2, D2], F32)
        nc.sync.dma_start(out=ones_sel, in_=osel[:, :, :])

        wloc_sb = consts.tile([D2, E], DT_IN)
        nc.sync.dma_start(out=wloc_sb, in_=wloc[:, :])

        attnT = consts.tile([D2, T], DT_IN)  # both heads' attn^T (f-major)

        for h in range(2):
            qrow = slice(0, 64) if h == 0 else slice(64, 128)
            drow = 64 if h == 0 else 32

            # --- loads ---
            qf_sb = io.tile([D2, T], F32, tag="qf")
            nc.sync.dma_start(out=qf_sb, in_=qf[h])
            kf_sb = io.tile([D2, T], F32, tag="kf")
            nc.sync.dma_start(out=kf_sb, in_=kf[h])
            kt_sb = io.tile([SC, NSC, D2], F32, tag="kt")
            nc.sync.dma_start(out=kt_sb, in_=kt[h].rearrange("s j d -> j s d"))
            # va host-padded to 256 cols per sub-chunk so the state matmul rhs
            # is N=256 (fp32r full rate); cols 128-255 are zero
            va_sb = io.tile([SC, NSC, 256], DT_IN, tag="va")
            nc.sync.dma_start(out=va_sb, in_=va[h].rearrange("s j d -> j s d"))

            # --- relu (q/k only; v passes through raw) ---
            Qr = work.tile([D2, T], DT_IN, tag="Qr")
            nc.scalar.activation(Qr, qf_sb, mybir.ActivationFunctionType.Relu)
            Kr = work.tile([D2, T], DT_IN, tag="Kr")
            nc.vector.tensor_scalar_max(Kr, kf_sb, 0.0)
            Ktr = work.tile([SC, NSC, D2], DT_IN, tag="Ktr")
            nc.scalar.activation(Ktr, kt_sb, mybir.ActivationFunctionType.Relu)

            qkvT = work.tile([D2, T], F32, tag="qkvT")
            psS = psum.tile([D2, 256], F32, tag="psS")
            S_sb = None

            for cc in range(NCH):
                lcols = slice(cc * C, (cc + 1) * C)
                # scores: two (128 x 256) strips side by side in one bank
                psA = psum.tile([SC, 512], F32, tag="psA")
                for s2 in range(2):
                    sc = 2 * cc + s2
                    nc.tensor.matmul(
                        psA[:, s2 * 256:(s2 + 1) * 256],
                        _mm(Kr[:, sc * SC:(sc + 1) * SC]),
                        _mm(Qr[:, lcols]),
                        start=True, stop=True,
                    )
                ATm = work.tile([SC, 512], DT_IN, tag="ATm")
                nc.vector.tensor_mul(ATm, psA, maskc)

                # qkv^T (+denom row drow) for this chunk
                psO = psum.tile([D2, C], F32, tag="psO")
                n_mm = 3 if cc > 0 else 2
                for s2 in range(2):
                    sc = 2 * cc + s2
                    nc.tensor.matmul(
                        psO,
                        _mm(va_sb[:, sc, 0:D2]),
                        _mm(ATm[:, s2 * 256:(s2 + 1) * 256]),
                        start=(s2 == 0), stop=(s2 == n_mm - 1),
                    )
                if cc > 0:
                    nc.tensor.matmul(
                        psO, _mm(S_sb), _mm(Qr[:, lcols]), start=False, stop=True,
                    )

                # state update for the *next* chunk
                if cc < NCH - 1:
                    for s2 in range(2):
                        sc = 2 * cc + s2
                        nc.tensor.matmul(
                            psS,
                            _mm(Ktr[:, sc, :]),
                            _mm(va_sb[:, sc, :]),
                            start=(cc == 0 and s2 == 0),
                            stop=(cc == NCH - 2 and s2 == 1),
                            skip_group_check=True,
                        )
                    S_sb = work.tile([D2, D2], DT_IN, tag="S_sb")
                    nc.scalar.copy(S_sb, psS[:, 0:D2])

                nc.scalar.copy(qkvT[:, lcols], psO)

            # --- divide by denominator (row drow of qkvT) ---
            dmax = work.tile([1, T], F32, tag="dmax")
            nc.vector.tensor_scalar_max(dmax, qkvT[drow:drow + 1, :], EPS)
            rr = work.tile([1, T], F32, tag="rr")
            nc.vector.reciprocal_approx_fast(out=rr, in_=dmax)
            for half in range(2):
                hcols = slice(half * 512, (half + 1) * 512)
                psB = psum.tile([D2, 512], F32, tag="ps512")
                nc.tensor.matmul(
                    psB, _mm(ones_sel[:, h, :]), _mm(rr[:, hcols]),
                    start=True, stop=True,
                )
                nc.vector.tensor_mul(
                    attnT[qrow, hcols], qkvT[qrow, hcols], psB[qrow, :],
                )

        # --- partial out-projection: out^T[e, t] = sum_f wloc[f, e] attnT[f, t] ---
        for ec in range(4):
            ot = work.tile([SC, T], F32, tag="ot")
            for th in range(2):
                psW = psum.tile([SC, 512], F32, tag="ps512")
                nc.tensor.matmul(
                    psW,
                    _mm(wloc_sb[:, ec * SC:(ec + 1) * SC]),
                    _mm(attnT[:, th * 512:(th + 1) * 512]),
                    start=True, stop=True,
                )
                if th == 0:
                    nc.vector.tensor_copy(ot[:, th * 512:(th + 1) * 512], psW)
                else:
                    nc.scalar.copy(ot[:, th * 512:(th + 1) * 512], psW)
            nc.sync.dma_start(out=outp[ec * SC:(ec + 1) * SC, :], in_=ot)

    nc.finalize()
    return nc


_NC_CACHE = {}


def _get_nc():
    key = MM_DT
    if key not in _NC_CACHE:
        _NC_CACHE[key] = _build_bass()
    return _NC_CACHE[key]


def _host_prep(q, k, v):
    """Build per-core input shards (layout/constant-scale prep only)."""
    idx = (np.float32(np.pi / 2) * np.arange(1, T + 1, dtype=np.float32)) / np.float32(T)
    s = np.sin(idx).astype(np.float32)
    c = np.cos(idx).astype(np.float32)

    q3 = q.reshape(NH, T, HID)
    k3 = k.reshape(NH, T, HID)
    vr = v.reshape(T, NH, HID).transpose(1, 0, 2)  # faithful buffer reinterpret

    qT = q3.transpose(0, 2, 1)
    kT = k3.transpose(0, 2, 1)
    qf = np.concatenate([qT * s[None, None, :], qT * c[None, None, :]], axis=1)
    kf = np.concatenate([kT * s[None, None, :], kT * c[None, None, :]], axis=1)
    kt = np.concatenate([k3 * s[:, None], k3 * c[:, None]], axis=-1)
    kt = np.ascontiguousarray(kt.reshape(NH, NSC, SC, D2))

    va = np.zeros((NH, NSC, SC, 256), np.float32)
    vch = vr.reshape(NH, NSC, SC, HID)
    va[0::2, :, :, 0:64] = vch[0::2]
    va[0::2, :, :, 64] = 1.0
    va[1::2, :, :, 32] = 1.0
    va[1::2, :, :, 64:128] = vch[1::2]

    qf = np.ascontiguousarray(qf.reshape(CORES, 2, D2, T))
    kf = np.ascontiguousarray(kf.reshape(CORES, 2, D2, T))
    kt = kt.reshape(CORES, 2, NSC, SC, D2)
    va = va.reshape(CORES, 2, NSC, SC, 256)
    return qf, kf, kt, va


def _assemble(results, W_out, b_out, q_dtype):
    ctx = np.zeros((N, H, T, HID), np.float32)
    for nb in range(N):
        outT = np.zeros((E, T), np.float32)
        for core in range(nb * 4, nb * 4 + 4):
            outT += results[core]["outp"]
        out = outT.T + b_out[None, :]
        ctx[nb] = out.reshape(T, H, HID).transpose(1, 0, 2)
    probs = np.zeros((N, H, T, T), np.float32)
    return ctx.astype(q_dtype, copy=False), probs


def run(q, k, v, W_out, b_out, trace=False, **spmd_kwargs):
    q = np.asarray(q, np.float32)
    k = np.asarray(k, np.float32)
    v = np.asarray(v, np.float32)
    W_out = np.asarray(W_out, np.float32)
    b_out = np.asarray(b_out, np.float32)

    qf, kf, kt, va = _host_prep(q, k, v)
    osel_np = np.zeros((1, 2, D2), np.float32)
    osel_np[0, 0, 0:64] = 1.0
    osel_np[0, 1, 64:128] = 1.0
    in_maps = []
    for core in range(CORES):
        fr = (core % 4) * D2
        in_maps.append({
            "qf": np.ascontiguousarray(qf[core]),
            "kf": np.ascontiguousarray(kf[core]),
            "kt": np.ascontiguousarray(kt[core]),
            "va": np.ascontiguousarray(va[core]),
            "wloc": np.ascontiguousarray(W_out[:, fr:fr + D2].T),
            "osel": osel_np,
        })

    nc = _get_nc()
    res = run_bass_kernel_spmd(nc, in_maps, list(range(CORES)), trace=trace, **spmd_kwargs)
    outputs = _assemble(res.results, W_out, b_out, np.float32)
    return outputs, res


def kernel(q, k, v, W_out, b_out):
    outputs, _ = run(q, k, v, W_out, b_out, trace=False)
    return outputs


# revision 16
# speedup vs baseline: 1.9246x; 1.0529x over previous
"""CosFormer attention (causal, relu, out_proj) — Trainium2 Bass kernel, 8-core SPMD.

Sharding: the 16 (batch*head) slices are split 2-per-core across 8 cores
(cores 0-3 -> batch 0 heads 0-7, cores 4-7 -> batch 1). Each core runs a
chunked linear-attention scan (chunk=256, sub-chunk=128) for its 2 heads plus
a partial out-projection over its 128 local features; the host sums the 4
partials per batch and adds the bias (pure unshard work).

Device math per head (feature-major layouts, l/j = time, d = 2*HID feature):
  Qf[d,l] = qT*sin / qT*cos (host-scaled const diag), relu on device -> Q
  A^T[j,l] = K^T Q  (per 128-row strip), causal-masked on DVE
  qkv^T(+denom row) = va^T @ A^T_masked (+ S^T_prev via state matmuls)
  attn^T = qkv^T * bcast(1/denom);  out^T[e,t] partial via wloc^T matmuls

The two heads are interleaved chunk-by-chunk so PE/DVE/ACT/DMA overlap.
Matmul-feeding tensors are float32r (1 cyc/row) — the BIR verifier requires
their producers to emit float32r.
"""

import os
from contextlib import ExitStack

import numpy as np

import concourse.bass as bass
import concourse.tile as tile
from concourse import bacc
from concourse import mybir
from concourse.bass_utils import run_bass_kernel_spmd
from concourse.masks import make_upper_triangular

N, H, T, HID = 2, 8, 1024, 64
NH = N * H
D2 = 2 * HID          # 128 feature dim after sin/cos concat
C = 256               # chunk length
SC = 128              # sub-chunk (matmul strip) length
NCH = T // C          # 4 chunks per head
NSC = T // SC         # 8 sub-chunks per head
E = 512               # embed dim
EPS = 1e-6
CORES = 8

F32 = mybir.dt.float32
F32R = mybir.dt.float32r

MM_DT = os.environ.get("COS_MM_DT", "f32r")
DT_IN = F32R if MM_DT == "f32r" else F32


def _build_bass():
    nc = bacc.Bacc(None, target_bir_lowering=False)

    qf = nc.dram_tensor("qf", (2, D2, T), F32, kind="ExternalInput")
    kf = nc.dram_tensor("kf", (2, D2, T), F32, kind="ExternalInput")
    kt = nc.dram_tensor("kt", (2, NSC, SC, D2), F32, kind="ExternalInput")
    va = nc.dram_tensor("va", (2, NSC, SC, 256), DT_IN, kind="ExternalInput")
    wloc = nc.dram_tensor("wloc", (D2, E), DT_IN, kind="ExternalInput")
    outp = nc.dram_tensor("outp", (E, T), F32, kind="ExternalOutput")

    with ExitStack() as ctx:
        ctx.enter_context(nc.allow_low_precision(reason="fp32r matmul inputs"))
        tc = ctx.enter_context(tile.TileContext(nc))
        consts = ctx.enter_context(tc.tile_pool(name="consts", bufs=1))
        io = ctx.enter_context(tc.tile_pool(name="io", bufs=2))
        work = ctx.enter_context(tc.tile_pool(name="work", bufs=2))
        psum = ctx.enter_context(tc.tile_pool(name="psum", bufs=2, space="PSUM"))
        dram = ctx.enter_context(tc.tile_pool(name="dram", bufs=2, space="DRAM"))

        # causal mask for one chunk's two score strips:
        # cols 0-255: strip sc0 = [UT | ones];  cols 256-511: strip sc1 = [zeros | UT]
        maskc = consts.tile([SC, 512], F32)
        nc.gpsimd.memset(maskc[:, 128:256], 1.0)
        nc.gpsimd.memset(maskc[:, 256:384], 0.0)
        make_upper_triangular(nc, maskc[:, 0:128], val=1.0, diag=True)
        make_upper_triangular(nc, maskc[:, 384:512], val=1.0, diag=True)

        attnT = consts.tile([D2, T], DT_IN)  # both heads' attn^T (f-major)
        rb = consts.tile([D2, T], F32)       # broadcast 1/denom rows, per-head halves

        heads = []
        for h in range(2):
            qf_sb = io.tile([D2, T], F32, tag="qf", name=f"qf_sb{h}")
            nc.sync.dma_start(out=qf_sb, in_=qf[h])
            kf_sb = io.tile([D2, T], F32, tag="kf", name=f"kf_sb{h}")
            nc.sync.dma_start(out=kf_sb, in_=kf[h])
            kt_sb = io.tile([SC, NSC, D2], F32, tag="kt", name=f"kt_sb{h}")
            nc.sync.dma_start(out=kt_sb, in_=kt[h].rearrange("s j d -> j s d"))
            # va host-padded to 256 cols per sub-chunk so the state matmul rhs
            # is N=256 (fp32r full rate); cols 128-255 are zero
            va_sb = io.tile([SC, NSC, 256], DT_IN, tag="va", name=f"va_sb{h}")
            nc.sync.dma_start(out=va_sb, in_=va[h].rearrange("s j d -> j s d"))

            # relu for q/k (v passes through raw)
            Qr = work.tile([D2, T], DT_IN, tag="Qr", name=f"Qr{h}")
            nc.scalar.activation(Qr, qf_sb, mybir.ActivationFunctionType.Relu)
            Kr = work.tile([D2, T], DT_IN, tag="Kr", name=f"Kr{h}")
            nc.vector.tensor_scalar_max(Kr, kf_sb, 0.0)
            Ktr = work.tile([SC, NSC, D2], DT_IN, tag="Ktr", name=f"Ktr{h}")
            nc.scalar.activation(Ktr, kt_sb, mybir.ActivationFunctionType.Relu)

            heads.append({
                "Qr": Qr, "Kr": Kr, "Ktr": Ktr, "va_sb": va_sb,
                "qkvT": work.tile([D2, T], F32, tag="qkvT", name=f"qkvT{h}"),
                "psS": psum.tile([D2, 256], F32, tag="psS", name=f"psS{h}"),
                "S_sb": None,
                "qrow": slice(0, 64) if h == 0 else slice(64, 128),
                "drow": 64 if h == 0 else 32,
            })

        wloc_sb = consts.tile([D2, E], DT_IN)
        nc.sync.dma_start(out=wloc_sb, in_=wloc[:, :])

        # chunk-interleaved scan over both heads
        for cc in range(NCH):
            lcols = slice(cc * C, (cc + 1) * C)
            for h in range(2):
                hd = heads[h]
                Qr, Kr, Ktr, va_sb = hd["Qr"], hd["Kr"], hd["Ktr"], hd["va_sb"]
                psA = psum.tile([SC, 512], F32, tag="psA", name=f"psA{h}_{cc}")
                for s2 in range(2):
                    sc = 2 * cc + s2
                    nc.tensor.matmul(
                        psA[:, s2 * 256:(s2 + 1) * 256],
                        Kr[:, sc * SC:(sc + 1) * SC],
                        Qr[:, lcols],
                        start=True, stop=True,
                    )
                ATm = work.tile([SC, 512], DT_IN, tag="ATm", name=f"ATm{h}_{cc}")
                nc.vector.tensor_mul(ATm, psA, maskc)

                psO = psum.tile([D2, C], F32, tag="psO", name=f"psO{h}_{cc}")
                n_mm = 3 if cc > 0 else 2
                for s2 in range(2):
                    sc = 2 * cc + s2
                    nc.tensor.matmul(
                        psO,
                        va_sb[:, sc, 0:D2],
                        ATm[:, s2 * 256:(s2 + 1) * 256],
                        start=(s2 == 0), stop=(s2 == n_mm - 1),
                    )
                if cc > 0:
                    nc.tensor.matmul(
                        psO, hd["S_sb"], Qr[:, lcols], start=False, stop=True,
                    )

                if cc < NCH - 1:
                    for s2 in range(2):
                        sc = 2 * cc + s2
                        nc.tensor.matmul(
                            hd["psS"],
                            Ktr[:, sc, :],
                            va_sb[:, sc, :],
                            start=(cc == 0 and s2 == 0),
                            stop=(cc == NCH - 2 and s2 == 1),
                            skip_group_check=True,
                        )
                    S_sb = work.tile([D2, D2], DT_IN, tag="S_sb", bufs=4,
                                     name=f"S{h}_{cc}")
                    nc.scalar.copy(S_sb, hd["psS"][:, 0:D2])
                    hd["S_sb"] = S_sb

                nc.scalar.copy(hd["qkvT"][:, lcols], psO)

        # per-head: 1/denominator, partition-broadcast via a DRAM round-trip
        # (SBUF APs can't have stride-0 partitions; DRAM sources can)
        for h in range(2):
            hd = heads[h]
            qrow, drow, qkvT = hd["qrow"], hd["drow"], hd["qkvT"]
            dmax = work.tile([1, T], F32, tag="dmax", name=f"dmax{h}")
            nc.vector.tensor_scalar_max(dmax, qkvT[drow:drow + 1, :], EPS)
            rr = work.tile([1, T], F32, tag="rr", name=f"rr{h}")
            nc.vector.reciprocal_approx_fast(out=rr, in_=dmax)
            rrd = dram.tile([1, T], F32, tag="rrd", name=f"rrd{h}")
            nc.sync.dma_start(out=rrd, in_=rr)
            rr_bcast = bass.AP(
                tensor=rrd.tensor, offset=rrd.offset,
                ap=[[0, 64], [1, T]],
            )
            nc.sync.dma_start(out=rb[qrow, :], in_=rr_bcast)
            nc.vector.tensor_mul(attnT[qrow, :], qkvT[qrow, :], rb[qrow, :])

        # partial out-projection: out^T[e, t] = sum_f wloc[f, e] attnT[f, t]
        for ec in range(4):
            ot = work.tile([SC, T], F32, tag="ot", name=f"ot{ec}")
            for th in range(2):
                psW = psum.tile([SC, 512], F32, tag="ps512", name=f"psW{ec}_{th}")
                nc.tensor.matmul(
                    psW,
                    wloc_sb[:, ec * SC:(ec + 1) * SC],
                    attnT[:, th * 512:(th + 1) * 512],
                    start=True, stop=True,
                )
                if th == 0:
                    nc.vector.tensor_copy(ot[:, th * 512:(th + 1) * 512], psW)
                else:
                    nc.scalar.copy(ot[:, th * 512:(th + 1) * 512], psW)
            nc.sync.dma_start(out=outp[ec * SC:(ec + 1) * SC, :], in_=ot)

    nc.finalize()
    return nc


_NC_CACHE = {}


def _get_nc():
    key = MM_DT
    if key not in _NC_CACHE:
        _NC_CACHE[key] = _build_bass()
    return _NC_CACHE[key]


def _host_prep(q, k, v):
    """Build per-core input shards (layout/constant-scale prep only)."""
    idx = (np.float32(np.pi / 2) * np.arange(1, T + 1, dtype=np.float32)) / np.float32(T)
    s = np.sin(idx).astype(np.float32)
    c = np.cos(idx).astype(np.float32)

    q3 = q.reshape(NH, T, HID)
    k3 = k.reshape(NH, T, HID)
    vr = v.reshape(T, NH, HID).transpose(1, 0, 2)  # faithful buffer reinterpret

    qT = q3.transpose(0, 2, 1)
    kT = k3.transpose(0, 2, 1)
    qf = np.concatenate([qT * s[None, None, :], qT * c[None, None, :]], axis=1)
    kf = np.concatenate([kT * s[None, None, :], kT * c[None, None, :]], axis=1)
    kt = np.concatenate([k3 * s[:, None], k3 * c[:, None]], axis=-1)
    kt = np.ascontiguousarray(kt.reshape(NH, NSC, SC, D2))

    va = np.zeros((NH, NSC, SC, 256), np.float32)
    vch = vr.reshape(NH, NSC, SC, HID)
    va[0::2, :, :, 0:64] = vch[0::2]
    va[0::2, :, :, 64] = 1.0
    va[1::2, :, :, 32] = 1.0
    va[1::2, :, :, 64:128] = vch[1::2]

    qf = np.ascontiguousarray(qf.reshape(CORES, 2, D2, T))
    kf = np.ascontiguousarray(kf.reshape(CORES, 2, D2, T))
    kt = kt.reshape(CORES, 2, NSC, SC, D2)
    va = va.reshape(CORES, 2, NSC, SC, 256)
    return qf, kf, kt, va


def _assemble(results, W_out, b_out, q_dtype):
    ctx = np.zeros((N, H, T, HID), np.float32)
    for nb in range(N):
        outT = np.zeros((E, T), np.float32)
        for core in range(nb * 4, nb * 4 + 4):
            outT += results[core]["outp"]
        out = outT.T + b_out[None, :]
        ctx[nb] = out.reshape(T, H, HID).transpose(1, 0, 2)
    probs = np.zeros((N, H, T, T), np.float32)
    return ctx.astype(q_dtype, copy=False), probs


def run(q, k, v, W_out, b_out, trace=False, **spmd_kwargs):
    q = np.asarray(q, np.float32)
    k = np.asarray(k, np.float32)
    v = np.asarray(v, np.float32)
    W_out = np.asarray(W_out, np.float32)
    b_out = np.asarray(b_out, np.float32)

    qf, kf, kt, va = _host_prep(q, k, v)
    in_maps = []
    for core in range(CORES):
        fr = (core % 4) * D2
        in_maps.append({
            "qf": np.ascontiguousarray(qf[core]),
            "kf": np.ascontiguousarray(kf[core]),
            "kt": np.ascontiguousarray(kt[core]),
            "va": np.ascontiguousarray(va[core]),
            "wloc": np.ascontiguousarray(W_out[:, fr:fr + D2].T),
        })

    nc = _get_nc()
    res = run_bass_kernel_spmd(nc, in_maps, list(range(CORES)), trace=trace, **spmd_kwargs)
    outputs = _assemble(res.results, W_out, b_out, np.float32)
    return outputs, res


def kernel(q, k, v, W_out, b_out):
    outputs, _ = run(q, k, v, W_out, b_out, trace=False)
    return outputs


# revision 18
# speedup vs baseline: 2.1257x; 1.1045x over previous
"""CosFormer attention (causal, relu, out_proj) — Trainium2 Bass kernel, 8-core SPMD.

Sharding: the 16 (batch*head) slices are split 2-per-core across 8 cores
(cores 0-3 -> batch 0 heads 0-7, cores 4-7 -> batch 1). Each core runs a
chunked linear-attention scan (chunk=256, sub-chunk=128) for its 2 heads plus
a partial out-projection over its 128 local features; the host sums the 4
partials per batch and adds the bias (pure unshard work).

Device math per head (feature-major layouts, l/j = time, d = 2*HID feature):
  Qf[d,l] = qT*sin / qT*cos (host-scaled const diag), relu on device -> Q
  A^T[j,l] = K^T Q  (per 128-row strip), causal-masked on DVE
  qkv^T(+denom row) = va^T @ A^T_masked (+ S^T_prev via state matmuls)
  attn^T = qkv^T * bcast(1/denom);  out^T[e,t] partial via wloc^T matmuls

The two heads are interleaved chunk-by-chunk so PE/DVE/ACT/DMA overlap.
Matmul-feeding tensors are float32r (1 cyc/row) — the BIR verifier requires
their producers to emit float32r.
"""

import os
from contextlib import ExitStack

import numpy as np

import concourse.bass as bass
import concourse.tile as tile
from concourse import bacc
from concourse import mybir
from concourse.bass_utils import run_bass_kernel_spmd
from concourse.masks import make_upper_triangular

N, H, T, HID = 2, 8, 1024, 64
NH = N * H
D2 = 2 * HID          # 128 feature dim after sin/cos concat
C = 256               # chunk length
SC = 128              # sub-chunk (matmul strip) length
NCH = T // C          # 4 chunks per head
NSC = T // SC         # 8 sub-chunks per head
E = 512               # embed dim
EPS = 1e-6
CORES = 8

F32 = mybir.dt.float32
F32R = mybir.dt.float32r

MM_DT = os.environ.get("COS_MM_DT", "f32r")
DT_IN = F32R if MM_DT == "f32r" else F32


def _build_bass():
    nc = bacc.Bacc(None, target_bir_lowering=False)

    qf = nc.dram_tensor("qf", (2, D2, T), F32, kind="ExternalInput")
    kf = nc.dram_tensor("kf", (2, D2, T), F32, kind="ExternalInput")
    kt = nc.dram_tensor("kt", (2, NSC, SC, D2), F32, kind="ExternalInput")
    va = nc.dram_tensor("va", (2, NSC, SC, D2), DT_IN, kind="ExternalInput")
    osel = nc.dram_tensor("osel", (1, 2, D2), DT_IN, kind="ExternalInput")
    wloc = nc.dram_tensor("wloc", (D2, E), DT_IN, kind="ExternalInput")
    outp = nc.dram_tensor("outp", (E, T), F32, kind="ExternalOutput")

    with ExitStack() as ctx:
        ctx.enter_context(nc.allow_low_precision(reason="fp32r matmul inputs"))
        tc = ctx.enter_context(tile.TileContext(nc))
        consts = ctx.enter_context(tc.tile_pool(name="consts", bufs=1))
        io = ctx.enter_context(tc.tile_pool(name="io", bufs=2))
        work = ctx.enter_context(tc.tile_pool(name="work", bufs=2))
        psum = ctx.enter_context(tc.tile_pool(name="psum", bufs=2, space="PSUM"))

        # causal mask for one chunk's two score strips:
        # cols 0-255: strip sc0 = [UT | ones];  cols 256-511: strip sc1 = [zeros | UT]
        maskc = consts.tile([SC, 512], F32)
        nc.gpsimd.memset(maskc[:, 128:256], 1.0)
        nc.gpsimd.memset(maskc[:, 256:384], 0.0)
        make_upper_triangular(nc, maskc[:, 0:128], val=1.0, diag=True)
        make_upper_triangular(nc, maskc[:, 384:512], val=1.0, diag=True)

        attnT = consts.tile([D2, T], DT_IN)  # both heads' attn^T (f-major)
        ones_sel = consts.tile([1, 2, D2], DT_IN)
        nc.sync.dma_start(out=ones_sel, in_=osel[:, :, :])

        heads = []
        for h in range(2):
            qf_sb = io.tile([D2, T], F32, tag="qf", name=f"qf_sb{h}")
            nc.sync.dma_start(out=qf_sb, in_=qf[h])
            kf_sb = io.tile([D2, T], F32, tag="kf", name=f"kf_sb{h}")
            nc.sync.dma_start(out=kf_sb, in_=kf[h])
            va_sb = io.tile([SC, NSC, D2], DT_IN, tag="va", name=f"va_sb{h}")
            nc.sync.dma_start(out=va_sb, in_=va[h].rearrange("s j d -> j s d"))
            kt_sb = io.tile([SC, NSC, D2], F32, tag="kt", name=f"kt_sb{h}")
            nc.sync.dma_start(out=kt_sb, in_=kt[h].rearrange("s j d -> j s d"))

            # relu for q/k (v passes through raw)
            Qr = work.tile([D2, T], DT_IN, tag="Qr", name=f"Qr{h}")
            for hv in range(2):
                cols = slice(hv * 512, (hv + 1) * 512)
                nc.scalar.activation(Qr[:, cols], qf_sb[:, cols],
                                     mybir.ActivationFunctionType.Relu)
            Kr = work.tile([D2, T], DT_IN, tag="Kr", name=f"Kr{h}")
            for hv in range(2):
                cols = slice(hv * 512, (hv + 1) * 512)
                nc.vector.tensor_scalar_max(Kr[:, cols], kf_sb[:, cols], 0.0)
            Ktr = work.tile([SC, NSC, D2], DT_IN, tag="Ktr", name=f"Ktr{h}")
            nc.scalar.activation(Ktr, kt_sb, mybir.ActivationFunctionType.Relu)

            heads.append({
                "Qr": Qr, "Kr": Kr, "Ktr": Ktr, "va_sb": va_sb,
                "qkvT": work.tile([D2, T], F32, tag="qkvT", name=f"qkvT{h}"),
                "psS": psum.tile([D2, D2], F32, tag="psS", name=f"psS{h}"),
                "S_sb": None,
                "qrow": slice(0, 64) if h == 0 else slice(64, 128),
                "drow": 64 if h == 0 else 32,
            })

        wloc_sb = consts.tile([D2, E], DT_IN)
        nc.sync.dma_start(out=wloc_sb, in_=wloc[:, :])

        # chunk-interleaved scan over both heads
        for cc in range(NCH):
            lcols = slice(cc * C, (cc + 1) * C)
            for h in range(2):
                hd = heads[h]
                Qr, Kr, Ktr, va_sb = hd["Qr"], hd["Kr"], hd["Ktr"], hd["va_sb"]
                psA = psum.tile([SC, 512], F32, tag="psA", name=f"psA{h}_{cc}")
                for s2 in range(2):
                    sc = 2 * cc + s2
                    nc.tensor.matmul(
                        psA[:, s2 * 256:(s2 + 1) * 256],
                        Kr[:, sc * SC:(sc + 1) * SC],
                        Qr[:, lcols],
                        start=True, stop=True,
                    )
                ATm = work.tile([SC, 512], DT_IN, tag="ATm", name=f"ATm{h}_{cc}")
                nc.vector.tensor_mul(ATm, psA, maskc)

                psO = psum.tile([D2, C], F32, tag="psO", name=f"psO{h}_{cc}")
                n_mm = 3 if cc > 0 else 2
                for s2 in range(2):
                    sc = 2 * cc + s2
                    nc.tensor.matmul(
                        psO,
                        va_sb[:, sc, :],
                        ATm[:, s2 * 256:(s2 + 1) * 256],
                        start=(s2 == 0), stop=(s2 == n_mm - 1),
                    )
                if cc > 0:
                    nc.tensor.matmul(
                        psO, hd["S_sb"], Qr[:, lcols], start=False, stop=True,
                    )

                if cc < NCH - 1:
                    for s2 in range(2):
                        sc = 2 * cc + s2
                        nc.tensor.matmul(
                            hd["psS"],
                            Ktr[:, sc, :],
                            va_sb[:, sc, 0:D2],
                            start=(cc == 0 and s2 == 0),
                            stop=(cc == NCH - 2 and s2 == 1),
                            skip_group_check=True,
                        )
                    S_sb = work.tile([D2, D2], DT_IN, tag="S_sb", bufs=4,
                                     name=f"S{h}_{cc}")
                    nc.scalar.copy(S_sb, hd["psS"])
                    hd["S_sb"] = S_sb

                nc.scalar.copy(hd["qkvT"][:, lcols], psO)

                # epilogue as soon as this head's last chunk lands:
                # 1/denominator -> f32r broadcast matmul -> divide into attnT
                if cc == NCH - 1:
                    qrow, drow, qkvT = hd["qrow"], hd["drow"], hd["qkvT"]
                    dmax = work.tile([1, T], F32, tag="dmax", name=f"dmax{h}")
                    nc.vector.tensor_scalar_max(dmax, qkvT[drow:drow + 1, :], EPS)
                    rr = work.tile([1, T], F32, tag="rr", name=f"rr{h}")
                    nc.vector.reciprocal_approx_fast(out=rr, in_=dmax)
                    rr_r = work.tile([1, T], DT_IN, tag="rr_r", name=f"rr_r{h}")
                    nc.vector.tensor_copy(rr_r, rr)
                    for half in range(2):
                        hcols = slice(half * 512, (half + 1) * 512)
                        psB = psum.tile([D2, 512], F32, tag="ps512",
                                        name=f"psB{h}_{half}")
                        nc.tensor.matmul(
                            psB, ones_sel[:, h, :], rr_r[:, hcols],
                            start=True, stop=True,
                        )
                        nc.vector.tensor_mul(
                            attnT[qrow, hcols], qkvT[qrow, hcols], psB[qrow, :],
                        )

        # partial out-projection: out^T[e, t] = sum_f wloc[f, e] attnT[f, t]
        for ec in range(4):
            ot = work.tile([SC, T], F32, tag="ot", name=f"ot{ec}")
            for th in range(2):
                psW = psum.tile([SC, 512], F32, tag="ps512", name=f"psW{ec}_{th}")
                nc.tensor.matmul(
                    psW,
                    wloc_sb[:, ec * SC:(ec + 1) * SC],
                    attnT[:, th * 512:(th + 1) * 512],
                    start=True, stop=True,
                )
                if th == 0:
                    nc.vector.tensor_copy(ot[:, th * 512:(th + 1) * 512], psW)
                else:
                    nc.scalar.copy(ot[:, th * 512:(th + 1) * 512], psW)
            nc.sync.dma_start(out=outp[ec * SC:(ec + 1) * SC, :], in_=ot)

    nc.finalize()
    return nc


_NC_CACHE = {}


def _get_nc():
    key = MM_DT
    if key not in _NC_CACHE:
        _NC_CACHE[key] = _build_bass()
    return _NC_CACHE[key]


def _host_prep(q, k, v):
    """Build per-core input shards (layout/constant-scale prep only)."""
    idx = (np.float32(np.pi / 2) * np.arange(1, T + 1, dtype=np.float32)) / np.float32(T)
    s = np.sin(idx).astype(np.float32)
    c = np.cos(idx).astype(np.float32)

    q3 = q.reshape(NH, T, HID)
    k3 = k.reshape(NH, T, HID)
    vr = v.reshape(T, NH, HID).transpose(1, 0, 2)  # faithful buffer reinterpret

    qT = q3.transpose(0, 2, 1)
    kT = k3.transpose(0, 2, 1)
    qf = np.concatenate([qT * s[None, None, :], qT * c[None, None, :]], axis=1)
    kf = np.concatenate([kT * s[None, None, :], kT * c[None, None, :]], axis=1)
    kt = np.concatenate([k3 * s[:, None], k3 * c[:, None]], axis=-1)
    kt = np.ascontiguousarray(kt.reshape(NH, NSC, SC, D2))

    va = np.zeros((NH, NSC, SC, D2), np.float32)
    vch = vr.reshape(NH, NSC, SC, HID)
    va[0::2, :, :, 0:64] = vch[0::2]
    va[0::2, :, :, 64] = 1.0
    va[1::2, :, :, 32] = 1.0
    va[1::2, :, :, 64:128] = vch[1::2]

    qf = np.ascontiguousarray(qf.reshape(CORES, 2, D2, T))
    kf = np.ascontiguousarray(kf.reshape(CORES, 2, D2, T))
    kt = kt.reshape(CORES, 2, NSC, SC, D2)
    va = va.reshape(CORES, 2, NSC, SC, D2)
    return qf, kf, kt, va


def _assemble(results, W_out, b_out, q_dtype):
    ctx = np.zeros((N, H, T, HID), np.float32)
    for nb in range(N):
        outT = np.zeros((E, T), np.float32)
        for core in range(nb * 4, nb * 4 + 4):
            outT += results[core]["outp"]
        out = outT.T + b_out[None, :]
        ctx[nb] = out.reshape(T, H, HID).transpose(1, 0, 2)
    probs = np.zeros((N, H, T, T), np.float32)
    return ctx.astype(q_dtype, copy=False), probs


def run(q, k, v, W_out, b_out, trace=False, **spmd_kwargs):
    q = np.asarray(q, np.float32)
    k = np.asarray(k, np.float32)
    v = np.asarray(v, np.float32)
    W_out = np.asarray(W_out, np.float32)
    b_out = np.asarray(b_out, np.float32)

    qf, kf, kt, va = _host_prep(q, k, v)
    osel_np = np.zeros((1, 2, D2), np.float32)
    osel_np[0, 0, 0:64] = 1.0
    osel_np[0, 1, 64:128] = 1.0
    in_maps = []
    for core in range(CORES):
        fr = (core % 4) * D2
        in_maps.append({
            "qf": np.ascontiguousarray(qf[core]),
            "kf": np.ascontiguousarray(kf[core]),
            "kt": np.ascontiguousarray(kt[core]),
            "va": np.ascontiguousarray(va[core]),
            "wloc": np.ascontiguousarray(W_out[:, fr:fr + D2].T),
            "osel": osel_np,
        })

    nc = _get_nc()
    res = run_bass_kernel_spmd(nc, in_maps, list(range(CORES)), trace=trace, **spmd_kwargs)
    outputs = _assemble(res.results, W_out, b_out, np.float32)
    return outputs, res


def kernel(q, k, v, W_out, b_out):
    outputs, _ = run(q, k, v, W_out, b_out, trace=False)
    return outputs
